# revision 1
# baseline (speedup 1.0000x reference)
"""Trainium2 Bass kernel for nn_ColorConsistencyLoss (segment_reduce).

loss = 0.7 * mean_CE(log_softmax(output), target) + 0.3 * sigmoid(sum_l,c std(img_c * mask_l))

Strategy (8 NeuronCores, data-parallel over pixels). Per 128-pixel group g a
bf16 one-hot O[p,l] = (target_p == l) is built by DVE tensor_scalar is_equal.

CE gather term sum_p o[p, target_p] is computed WITHOUT the [L,L] segment-sum
matmul of the previous revision: one wide DVE multiply (o * onehot, all bf16
at 2x mode) per 4096-pixel macro-tile plus one wide row-sum reduce. This
removes the dominant PE cost (the 128-col stationary load + 100-col stream
per group); PE now only does the tiny per-label moment sums

    PSUM[0:6, l] += [img|img2]^T @ O      (lhsT = cat [128,6], rhs = onehot)

ACT computes exp(o) (f32->bf16); a DVE multi-group reduce produces per-pixel
softmax denominators s. No max-subtraction: inputs are ~N(0,1), exp can't
overflow. The device also finishes the per-pixel log: ACT Ln(s) with
accum_out gives sum_p log(s_p) per partition, so outputs are tiny:
lse_out [128,1], ce_out [128,1], st_out [6,L]. Host: CE=(sum lse - sum ce)/HW,
var/std/sigmoid on the [L,6] moments.

The o DMA is laid out so every SBUF partition reads a contiguous 12.8KB DRAM
run (pixel q = m*4096 + 32p + j -> partition p, group j): 128 large
descriptors per 1.6MB transfer. SWDGE (gpsimd) DMA casts f32->bf16 in-flight.
"""

import contextlib
import sys

for _p in ("/opt/trn_rl_repo", "/opt/trn_rl_repo/concourse"):
    if _p not in sys.path:
        sys.path.insert(0, _p)

import numpy as np

import concourse.bacc as bacc
import concourse.tile as tile
from concourse import mybir
from concourse.bass_utils import run_bass_kernel_spmd

# ---------------------------------------------------------------- constants
HW = 1048576          # total pixels
L = 100               # num labels (softmax width)
LP = 128              # one-hot padded width in consts (labels 100..127 never hit)
N_CORES = 8
PIX_PER_CORE = HW // N_CORES          # 131072
GPM = 32              # groups (of 128 pixels) per macro-tile
PIX_PER_MACRO = 128 * GPM             # 4096
N_MACROS = PIX_PER_CORE // PIX_PER_MACRO   # 32
ALPHA_SAL = 0.3

F32 = mybir.dt.float32
BF16 = mybir.dt.bfloat16
NP_BF16 = mybir.dt.np(BF16)


def build_nc(
    n_macros: int = N_MACROS,
    gpm: int = GPM,
    repeats: int = 1,
    do_ts: bool = True,
    do_mm: bool = True,
    do_exp: bool = True,
    do_ce: bool = True,
    ts_wide: bool = False,
    n_ts_dve: int = 32,
    prod_eng: str = "v",
    io_bufs: int = 2,
    oh_bufs: int = 3,
    staggered: bool = True,
):
    """Build the single-core Bass program (same program runs SPMD on all cores).

    repeats > 1 wraps the compute in an on-device For_i loop; used only for
    benchmarking. The do_* flags build timing-only ablation variants
    (results are wrong when a stage is disabled).

    ts_wide: build all gpm onehots in ONE tensor_tensor via broadcast APs.
    prod_eng: engine for the CE product ('v' vector, 'p' gpsimd, 'a' any).
    n_ts_dve: per-group onehots built on DVE; the rest go to gpsimd.
    """
    n_pix = 128 * gpm * n_macros
    n_groups_total = gpm * n_macros

    nc = bacc.Bacc("TRN2")

    n_const = LP + n_groups_total * 6
    o_d = nc.dram_tensor("o", [n_pix, L], F32, kind="ExternalInput")
    consts_d = nc.dram_tensor("consts", [128, n_const], BF16, kind="ExternalInput")
    tgtf_d = nc.dram_tensor("tgtf", [128, n_groups_total], F32, kind="ExternalInput")
    lse_d = nc.dram_tensor("lse_out", [128, 1], F32, kind="ExternalOutput")
    ce_d = nc.dram_tensor("ce_out", [128, 1], F32, kind="ExternalOutput")
    st_d = nc.dram_tensor("st_out", [6, L], F32, kind="ExternalOutput")

    # [n_pix, L] viewed as [n_macros, 128, gpm*L]; per partition the DRAM run
    # is contiguous (gpm consecutive pixel rows).
    o_view = o_d[:, :].rearrange("(m p j) e -> m p (j e)", p=128, j=gpm)

    prod_engine = {"v": nc.vector, "p": nc.gpsimd, "a": nc.any}[prod_eng]

    with tile.TileContext(nc) as tc:
        with (
            tc.tile_pool(name="consts", bufs=1) as cpool,
            tc.tile_pool(name="obuf", bufs=io_bufs) as opool,
            tc.tile_pool(name="ebuf", bufs=io_bufs) as epool,
            tc.tile_pool(name="prbuf", bufs=io_bufs) as prpool,
            tc.tile_pool(name="ohbuf", bufs=oh_bufs) as ohpool,
            tc.tile_pool(name="psum", bufs=1, space="PSUM") as ppool,
        ):
            consts_sb = cpool.tile([128, n_const], BF16)
            nc.sync.dma_start(out=consts_sb, in_=consts_d[:, :])
            tgt_sb = cpool.tile([128, n_groups_total], F32)
            nc.sync.dma_start(out=tgt_sb, in_=tgtf_d[:, :])
            iota_sb = consts_sb[:, 0:LP]
            cat_sb = consts_sb[:, LP:]
            s_sb = cpool.tile([128, n_groups_total], F32)
            ce_sb = cpool.tile([128, n_macros], F32)
            # Warm-up: each compute engine observes both const DMAs once, so
            # loop instructions don't each accumulate waits on the DMA sems.
            wu_v = cpool.tile([128, 1], F32)
            nc.vector.tensor_scalar(
                out=wu_v, in0=consts_sb[:, 0:1], scalar1=tgt_sb[:, 0:1],
                scalar2=None, op0=mybir.AluOpType.mult,
            )
            wu_s = cpool.tile([128, 1], BF16)
            nc.scalar.copy(out=wu_s, in_=consts_sb[:, 0:1])
            wu_s2 = cpool.tile([128, 1], F32)
            nc.scalar.copy(out=wu_s2, in_=tgt_sb[:, 0:1])
            wu_p = cpool.tile([128, 1], F32)
            nc.gpsimd.tensor_scalar(
                out=wu_p,
                in0=consts_sb[:, 0:1],
                scalar1=tgt_sb[:, 0:1],
                scalar2=None,
                op0=mybir.AluOpType.add,
            )

            st_ps = ppool.tile([6, L], F32)

            loop_cm = (
                tc.For_i(0, repeats, 1, staggered_reset=staggered)
                if repeats > 1
                else contextlib.nullcontext()
            )
            with loop_cm:
                for m in range(n_macros):
                    # SWDGE DMA casts f32 -> bf16 during the load (HBM reads
                    # are still the full f32 bytes; SBUF holds bf16).
                    o_t = opool.tile([128, gpm * L], BF16, tag="o")
                    nc.gpsimd.dma_start(out=o_t, in_=o_view[m])

                    # --- one-hots for the whole macro ---------------------
                    if do_ts:
                        oh = ohpool.tile([128, gpm * L], BF16, tag="oh")
                        if ts_wide:
                            iota_b = (
                                iota_sb[:, 0:L]
                                .unsqueeze(1)
                                .broadcast_to([128, gpm, L])
                            )
                            tgt_b = (
                                tgt_sb[:, m * gpm : (m + 1) * gpm]
                                .unsqueeze(2)
                                .broadcast_to([128, gpm, L])
                            )
                            nc.vector.tensor_tensor(
                                out=oh.rearrange("p (j e) -> p j e", e=L),
                                in0=iota_b,
                                in1=tgt_b,
                                op=mybir.AluOpType.is_equal,
                            )
                        else:
                            for j in range(gpm):
                                gidx = m * gpm + j
                                teng = nc.vector if j < n_ts_dve else nc.gpsimd
                                teng.tensor_scalar(
                                    out=oh[:, j * L : (j + 1) * L],
                                    in0=iota_sb[:, 0:L],
                                    scalar1=tgt_sb[:, gidx : gidx + 1],
                                    scalar2=None,
                                    op0=mybir.AluOpType.is_equal,
                                )
                    else:
                        oh = consts_sb[:, 0 : gpm * L]  # timing-only stand-in

                    # --- CE gather: sum_l onehot*o, then row-sum ----------
                    if do_ce:
                        prod = prpool.tile([128, gpm * L], BF16, tag="prod")
                        prod_engine.tensor_tensor(
                            out=prod, in0=o_t, in1=oh, op=mybir.AluOpType.mult
                        )
                        nc.vector.tensor_reduce(
                            out=ce_sb[:, m : m + 1],
                            in_=prod,
                            axis=mybir.AxisListType.X,
                            op=mybir.AluOpType.add,
                        )

                    # --- softmax denominators -----------------------------
                    if do_exp:
                        expo = epool.tile([128, gpm * L], BF16, tag="expo")
                        nc.scalar.activation(
                            out=expo, in_=o_t, func=mybir.ActivationFunctionType.Exp
                        )
                        nc.vector.tensor_reduce(
                            out=s_sb[:, m * gpm : (m + 1) * gpm],
                            in_=expo.rearrange("p (j e) -> p j e", e=L),
                            axis=mybir.AxisListType.X,
                            op=mybir.AluOpType.add,
                        )

                    # --- per-label moment sums on PE ----------------------
                    if do_mm:
                        for j in range(gpm):
                            gidx = m * gpm + j
                            nc.tensor.matmul(
                                st_ps,
                                lhsT=cat_sb[:, gidx * 6 : (gidx + 1) * 6],
                                rhs=oh[:, j * L : (j + 1) * L],
                                start=gidx == 0,
                                stop=gidx == n_groups_total - 1,
                            )

                    if not (do_exp or do_ts or do_mm or do_ce):
                        nc.vector.tensor_copy(
                            out=s_sb[:, m : m + 1], in_=o_t[:, 0:1]
                        )

                # --- finals ------------------------------------------------
                if do_exp:
                    lnj = cpool.tile([128, n_groups_total], BF16)
                    lse_sb = cpool.tile([128, 1], F32)
                    nc.scalar.activation(
                        out=lnj,
                        in_=s_sb,
                        func=mybir.ActivationFunctionType.Ln,
                        accum_out=lse_sb,
                    )
                    nc.sync.dma_start(out=lse_d[:, :], in_=lse_sb)
                if do_ce:
                    ce_tot = cpool.tile([128, 1], F32)
                    nc.vector.tensor_reduce(
                        out=ce_tot,
                        in_=ce_sb,
                        axis=mybir.AxisListType.X,
                        op=mybir.AluOpType.add,
                    )
                    nc.sync.dma_start(out=ce_d[:, :], in_=ce_tot)
                if do_mm:
                    st_sb = cpool.tile([6, L], F32)
                    nc.vector.tensor_copy(out=st_sb, in_=st_ps)
                    nc.sync.dma_start(out=st_d[:, :], in_=st_sb)
                if not (do_exp or do_ts or do_mm or do_ce):
                    nc.sync.dma_start(out=lse_d[:, :], in_=s_sb[:, 0:1])

    nc.compile()  # bacc lowering: splits >1-wait instructions for the TRN2 ISA
    return nc


def make_in_map(o_slice, tgt_slice, img_slice, n_macros: int = N_MACROS, gpm: int = GPM):
    """Host-side pre-layout for one core.

    o_slice   [n_pix, L] f32   (passed through as-is; device reshapes via AP)
    tgt_slice [n_pix]    int   -> tgt[p, m*gpm+j] = target[m*128*gpm + 32p + j]
    img_slice [n_pix, 3] f32   -> imgcat[p, (m*gpm+j)*6 + c] = [img, img^2]
    consts = bf16 [ iota(128) | imgcat ]
    """
    n_pix = 128 * gpm * n_macros
    assert o_slice.shape == (n_pix, L)

    t = tgt_slice.reshape(n_macros, 128, gpm)
    tgt_pre = np.ascontiguousarray(t.transpose(1, 0, 2)).reshape(128, n_macros * gpm)

    fl = img_slice.reshape(n_macros, 128, gpm, 3).astype(np.float32)
    cat = np.concatenate([fl, fl * fl], axis=-1)  # [m, p, j, 6]
    cat_pre = np.ascontiguousarray(cat.transpose(1, 0, 2, 3)).reshape(
        128, n_macros * gpm * 6
    )

    iota = np.broadcast_to(np.arange(LP, dtype=np.float32), (128, LP))
    consts = np.ascontiguousarray(
        np.concatenate([iota, cat_pre], axis=1).astype(NP_BF16)
    )
    return {
        "o": np.ascontiguousarray(o_slice, dtype=np.float32),
        "consts": consts,
        "tgtf": np.ascontiguousarray(tgt_pre.astype(np.float32)),
    }


def finalize(results, n_pix_total=HW):
    """Combine per-core partial results (host-side unshard) into the scalar loss."""
    lse_sum = 0.0
    gather_sum = 0.0
    s1 = np.zeros((L, 3), dtype=np.float64)
    s2 = np.zeros((L, 3), dtype=np.float64)
    for r in results:
        lse_sum += float(np.sum(np.asarray(r["lse_out"], dtype=np.float64)))
        gather_sum += float(np.sum(np.asarray(r["ce_out"], dtype=np.float64)))
        st = np.asarray(r["st_out"], dtype=np.float64)  # [6, 100]
        s1 += st[0:3, :].T
        s2 += st[3:6, :].T
    loss1 = (lse_sum - gather_sum) / n_pix_total
    mean = s1 / n_pix_total
    var = np.maximum(s2 / n_pix_total - mean * mean, 0.0)
    std_all = float(np.sum(np.sqrt(var)))
    loss2 = 1.0 / (1.0 + np.exp(-std_all))
    return np.float32((1.0 - ALPHA_SAL) * loss1 + ALPHA_SAL * loss2)


_NC_CACHE = {}


def _get_nc():
    if "nc" not in _NC_CACHE:
        _NC_CACHE["nc"] = build_nc()
    return _NC_CACHE["nc"]


def kernel(output, target, img):
    output = np.asarray(output, dtype=np.float32)
    target = np.asarray(target)
    img = np.asarray(img, dtype=np.float32)
    assert output.shape == (HW, L)
    img_flat = img.reshape(HW, 3)

    in_maps = []
    for c in range(N_CORES):
        lo, hi = c * PIX_PER_CORE, (c + 1) * PIX_PER_CORE
        in_maps.append(
            make_in_map(output[lo:hi], target[lo:hi], img_flat[lo:hi])
        )

    nc = _get_nc()
    res = run_bass_kernel_spmd(nc, in_maps, core_ids=list(range(N_CORES)))
    return finalize(res.results)


if __name__ == "__main__":
    nc = build_nc(n_macros=1)
    print("built ok:", len(nc.inst_map), "instructions")



# revision 6
# speedup vs baseline: 1.8861x; 1.8861x over previous
"""Trainium2 Bass kernel for nn_ColorConsistencyLoss (segment_reduce).

loss = 0.7 * mean_CE(log_softmax(output), target) + 0.3 * sigmoid(sum_l,c std(img_c * mask_l))

v3 strategy (8 NeuronCores, data-parallel over pixels; all heavy math bf16):

Host packs, per group g of 128 pixels, a stationary block
comb[128, 106] = [o (100 logits) | img (3) | img^2 (3)] (bf16). Per macro of
32 groups the device builds a label-major one-hot
    oh_t[p, l, j] = (tgt[p, j] == l)
in ONE wide tensor_tensor is_equal per engine chunk: in0 broadcasts the 32
targets across the label dim (innermost step stays +-1 so DVE runs 2x packed
mode); in1 is a host-shipped iota-replicated constant. The label range is
split DVE / gpsimd to balance engine load. One PE matmul per group

    PSUM[0:106, 0:100] += comb_g^T @ oh_t[:, :, j]      (strided rhs)

accumulated over all 1024 groups yields at once
  rows 0:100   -> trace = CE gather term sum_p o[p, t_p]
  rows 100:106 -> per-label (sum img, sum img^2) moments  [6, 100]
so the CE product and CE row-sum of older revisions vanish; host extracts
trace/moments from one [106,100] f32 output.

ACT computes exp(o) (one wide strided activation per macro). Softmax
denominators: DVE tensor_reduce has no packed mode (3394ns/macro), so two
bf16 tensor_tensor folds (100->50->25, 2x mode) + one short reduce do it in
2265ns. ACT Ln(s) with accum_out gives sum_p log s_p. Host: CE =
(sum lse - trace)/HW, var/std/sigmoid on the moments.

comb DMA is plain HWDGE (host ships bf16; halves HBM bytes vs f32+cast);
each partition reads one contiguous 6.8KB run per macro.
"""

import contextlib
import sys

for _p in ("/opt/trn_rl_repo", "/opt/trn_rl_repo/concourse"):
    if _p not in sys.path:
        sys.path.insert(0, _p)

import numpy as np

import concourse.bacc as bacc
import concourse.tile as tile
from concourse import mybir
from concourse.bass_utils import run_bass_kernel_spmd

# ---------------------------------------------------------------- constants
HW = 1048576          # total pixels
L = 100               # num labels (softmax width)
K = 106               # comb row width: 100 logits + img(3) + img^2(3)
N_CORES = 8
PIX_PER_CORE = HW // N_CORES          # 131072
GPM = 32              # groups (of 128 pixels) per macro-tile
PIX_PER_MACRO = 128 * GPM             # 4096
N_MACROS = PIX_PER_CORE // PIX_PER_MACRO   # 32
ALPHA_SAL = 0.3

F32 = mybir.dt.float32
BF16 = mybir.dt.bfloat16
NP_BF16 = mybir.dt.np(BF16)


def build_nc(
    n_macros: int = N_MACROS,
    gpm: int = GPM,
    repeats: int = 1,
    oh_dve_labels: int = 100,
    fold1_pool: bool = True,
    io_bufs: int = 4,
    oh_bufs: int = 4,
    ex_bufs: int = 4,
    staggered: bool = True,
):
    """Build the single-core Bass program (same program runs SPMD on all cores).

    repeats > 1 wraps the compute in an on-device For_i loop (benchmarking).
    oh_dve_labels: one-hot label rows built on DVE; the rest go to gpsimd.
    """
    n_groups_total = gpm * n_macros
    a = oh_dve_labels

    nc = bacc.Bacc("TRN2")

    comb_d = nc.dram_tensor("comb", [n_macros * 128, gpm * K], BF16, kind="ExternalInput")
    iotarep_d = nc.dram_tensor("iotarep", [128, L * gpm], BF16, kind="ExternalInput")
    tgtf_d = nc.dram_tensor("tgtf", [128, n_groups_total], BF16, kind="ExternalInput")
    lse_d = nc.dram_tensor("lse_out", [128, 1], F32, kind="ExternalOutput")
    st_d = nc.dram_tensor("st_out", [K, L], F32, kind="ExternalOutput")

    comb_view = comb_d[:, :].rearrange("(m p) e -> m p e", p=128)

    with tile.TileContext(nc) as tc:
        with (
            tc.tile_pool(name="consts", bufs=1) as cpool,
            tc.tile_pool(name="cbuf", bufs=io_bufs) as combpool,
            tc.tile_pool(name="ebuf", bufs=ex_bufs) as epool,
            tc.tile_pool(name="ohbuf", bufs=oh_bufs) as ohpool,
            tc.tile_pool(name="fbuf", bufs=2) as fpool,
            tc.tile_pool(name="psum", bufs=1, space="PSUM") as ppool,
        ):
            iotarep_sb = cpool.tile([128, L * gpm], BF16)
            nc.sync.dma_start(out=iotarep_sb, in_=iotarep_d[:, :])
            iotarep3 = iotarep_sb.rearrange("p (l j) -> p l j", j=gpm)
            tgt_sb = cpool.tile([128, n_groups_total], BF16)
            nc.sync.dma_start(out=tgt_sb, in_=tgtf_d[:, :])
            s_sb = cpool.tile([128, n_groups_total], BF16)
            # Warm-up: each compute engine observes both const DMAs once, so
            # loop instructions don't each accumulate waits on the DMA sems.
            wu_v = cpool.tile([128, 1], BF16)
            nc.vector.tensor_tensor(
                out=wu_v, in0=iotarep_sb[:, 0:1], in1=tgt_sb[:, 0:1],
                op=mybir.AluOpType.mult,
            )
            wu_s = cpool.tile([128, 1], BF16)
            nc.scalar.copy(out=wu_s, in_=iotarep_sb[:, 0:1])
            wu_s2 = cpool.tile([128, 1], BF16)
            nc.scalar.copy(out=wu_s2, in_=tgt_sb[:, 0:1])
            wu_p = cpool.tile([128, 1], BF16)
            nc.gpsimd.tensor_tensor(
                out=wu_p, in0=iotarep_sb[:, 0:1], in1=tgt_sb[:, 0:1],
                op=mybir.AluOpType.add,
            )

            st_ps = ppool.tile([K, L], F32)

            loop_cm = (
                tc.For_i(0, repeats, 1, staggered_reset=staggered)
                if repeats > 1
                else contextlib.nullcontext()
            )
            with loop_cm:
                for m in range(n_macros):
                    comb_t = combpool.tile([128, gpm * K], BF16, tag="comb")
                    nc.sync.dma_start(out=comb_t, in_=comb_view[m])
                    comb_3d = comb_t.rearrange("p (j k) -> p j k", k=K)

                    # --- one-hot (label-major), DVE/gpsimd split ----------
                    oh_t = ohpool.tile([128, L * gpm], BF16, tag="oh")
                    oh3 = oh_t.rearrange("p (l j) -> p l j", j=gpm)
                    tgt_b = (
                        tgt_sb[:, m * gpm : (m + 1) * gpm]
                        .unsqueeze(1)
                        .broadcast_to([128, L, gpm])
                    )
                    if a > 0:
                        nc.vector.tensor_tensor(
                            out=oh3[:, 0:a, :], in0=tgt_b[:, 0:a, :],
                            in1=iotarep3[:, 0:a, :], op=mybir.AluOpType.is_equal,
                        )
                    if a < L:
                        nc.gpsimd.tensor_tensor(
                            out=oh3[:, a:L, :], in0=tgt_b[:, a:L, :],
                            in1=iotarep3[:, a:L, :], op=mybir.AluOpType.is_equal,
                        )

                    # --- softmax denominators -----------------------------
                    expo = epool.tile([128, gpm * L], BF16, tag="expo")
                    expo3 = expo.rearrange("p (j e) -> p j e", e=L)
                    nc.scalar.activation(
                        out=expo3,
                        in_=comb_3d[:, :, 0:L],
                        func=mybir.ActivationFunctionType.Exp,
                    )
                    h1 = fpool.tile([128, gpm * 50], BF16, tag="h1")
                    h13 = h1.rearrange("p (j e) -> p j e", e=50)
                    fold1_eng = nc.gpsimd if fold1_pool else nc.vector
                    fold1_eng.tensor_tensor(
                        out=h13, in0=expo3[:, :, 0:50], in1=expo3[:, :, 50:100],
                        op=mybir.AluOpType.add,
                    )
                    h2 = fpool.tile([128, gpm * 25], BF16, tag="h2")
                    h23 = h2.rearrange("p (j e) -> p j e", e=25)
                    nc.vector.tensor_tensor(
                        out=h23, in0=h13[:, :, 0:25], in1=h13[:, :, 25:50],
                        op=mybir.AluOpType.add,
                    )
                    with nc.allow_low_precision(
                        reason="bf16 softmax denominators; CE error averages "
                        "over 1M pixels"
                    ):
                        nc.vector.tensor_reduce(
                            out=s_sb[:, m * gpm : (m + 1) * gpm],
                            in_=h23,
                            axis=mybir.AxisListType.X,
                            op=mybir.AluOpType.add,
                        )

                    # --- gather + moments on PE ---------------------------
                    for j in range(gpm):
                        gidx = m * gpm + j
                        nc.tensor.matmul(
                            st_ps,
                            lhsT=comb_t[:, j * K : (j + 1) * K],
                            rhs=oh3[:, :, j],
                            start=gidx == 0,
                            stop=gidx == n_groups_total - 1,
                        )

                # --- finals ------------------------------------------------
                lnj = cpool.tile([128, n_groups_total], BF16)
                lse_sb = cpool.tile([128, 1], F32)
                nc.scalar.activation(
                    out=lnj,
                    in_=s_sb,
                    func=mybir.ActivationFunctionType.Ln,
                    accum_out=lse_sb,
                )
                nc.sync.dma_start(out=lse_d[:, :], in_=lse_sb)
                st_sb = cpool.tile([K, L], F32)
                nc.vector.tensor_copy(out=st_sb, in_=st_ps)
                nc.sync.dma_start(out=st_d[:, :], in_=st_sb)

    nc.compile()  # bacc lowering: splits >1-wait instructions for the TRN2 ISA
    return nc


def make_in_map(o_slice, tgt_slice, img_slice, n_macros: int = N_MACROS, gpm: int = GPM):
    """Host-side pre-layout for one core.

    Pixel q = m*(128*gpm) + p*gpm + j  ->  macro m, partition p, group j.
    comb[m*128+p, j*K + 0:100]   = o[q]        (bf16)
    comb[m*128+p, j*K + 100:103] = img[q]      (bf16)
    comb[m*128+p, j*K + 103:106] = img[q]^2    (bf16)
    tgtf[p, m*gpm+j] = target[q]               (bf16; labels < 256 exact)
    iotarep[p, l*gpm+j] = l                    (bf16 const)
    """
    n_pix = 128 * gpm * n_macros
    assert o_slice.shape == (n_pix, L)

    o4 = np.asarray(o_slice, dtype=np.float32).reshape(n_macros, 128, gpm, L)
    img4 = np.asarray(img_slice, dtype=np.float32).reshape(n_macros, 128, gpm, 3)
    comb = np.empty((n_macros, 128, gpm, K), dtype=np.float32)
    comb[..., 0:L] = o4
    comb[..., L : L + 3] = img4
    comb[..., L + 3 : K] = img4 * img4

    t = np.asarray(tgt_slice).reshape(n_macros, 128, gpm)
    tgt_pre = np.ascontiguousarray(t.transpose(1, 0, 2)).reshape(128, n_macros * gpm)

    iotarep = np.broadcast_to(
        np.repeat(np.arange(L, dtype=np.float32), gpm), (128, L * gpm)
    )
    return {
        "comb": comb.astype(NP_BF16).reshape(n_macros * 128, gpm * K),
        "iotarep": np.ascontiguousarray(iotarep.astype(NP_BF16)),
        "tgtf": np.ascontiguousarray(tgt_pre.astype(NP_BF16)),
    }


def finalize(results, n_pix_total=HW):
    """Combine per-core partial results (host-side unshard) into the scalar loss."""
    lse_sum = 0.0
    gather_sum = 0.0
    s1 = np.zeros((L, 3), dtype=np.float64)
    s2 = np.zeros((L, 3), dtype=np.float64)
    for r in results:
        lse_sum += float(np.sum(np.asarray(r["lse_out"], dtype=np.float64)))
        st = np.asarray(r["st_out"], dtype=np.float64)  # [106, 100]
        gather_sum += float(np.trace(st[0:L, 0:L]))
        s1 += st[L : L + 3, :].T
        s2 += st[L + 3 : K, :].T
    loss1 = (lse_sum - gather_sum) / n_pix_total
    mean = s1 / n_pix_total
    var = np.maximum(s2 / n_pix_total - mean * mean, 0.0)
    std_all = float(np.sum(np.sqrt(var)))
    loss2 = 1.0 / (1.0 + np.exp(-std_all))
    return np.float32((1.0 - ALPHA_SAL) * loss1 + ALPHA_SAL * loss2)


_NC_CACHE = {}


def _get_nc():
    if "nc" not in _NC_CACHE:
        _NC_CACHE["nc"] = build_nc()
    return _NC_CACHE["nc"]


def kernel(output, target, img):
    output = np.asarray(output, dtype=np.float32)
    target = np.asarray(target)
    img = np.asarray(img, dtype=np.float32)
    assert output.shape == (HW, L)
    img_flat = img.reshape(HW, 3)

    in_maps = []
    for c in range(N_CORES):
        lo, hi = c * PIX_PER_CORE, (c + 1) * PIX_PER_CORE
        in_maps.append(
            make_in_map(output[lo:hi], target[lo:hi], img_flat[lo:hi])
        )

    nc = _get_nc()
    res = run_bass_kernel_spmd(nc, in_maps, core_ids=list(range(N_CORES)))
    return finalize(res.results)


if __name__ == "__main__":
    nc = build_nc(n_macros=1)
    print("built ok:", len(nc.inst_map), "instructions")
